# revision 1
# baseline (speedup 1.0000x reference)
"""AttentionRNN Trainium2 kernel.

Problem: B=128, T=512, H=1024, V=128
  xe = Wxh[x]                               (gather == onehot(x) @ Wxh)
  h_t = tanh(xe_t + h_{t-1} @ Whh + bh)     (512 sequential steps)
  S   = Hs @ Hs^T  (per batch);  W = softmax(S, axis=-1)
  ctx = W @ Hs;    out = [Hs, ctx] @ fc_w.T + fc_b

Sharding: data-parallel over batch, 16 batches per core, 8 cores. Params
replicated. No collectives.

Key design points:
 - Recurrence state kept hidden-major (hT: [128 h-part, 8k x 32] cols, 16
   real batch cols + 16 zero-pad per k-chunk). Per step, z = h@Whh + onehot@Wxh'
   computed batch-major via 4 PE column-groups (tile_position=(0,32g)), each
   group computing a 256-wide n-slice with the FULL contraction (8 Whh k-chunks
   + vocab chunk) in fp32r -> no cross-group reduction.
 - tanh in ONE ACT instr over [128,256] (pad rows are zeros).
 - h transposed back to hidden-major with 8 PE transposes -> hT (fp32) and
   appended to HsT (bf16) for the attention phase.
 - Attention per batch: scores via HsT (bf16), exp WITHOUT max-subtraction
   (P = exp(S) is then symmetric since S is), row sums via ACT accum_out.
 - ASSOCIATIVITY: ctx @ fc_wc.T = P @ (Hs @ fc_wc.T) = P @ G. G is [512,128],
   so no context materialization, no transposes, no DRAM streaming.
   P^T blocks needed as lhsT are read directly from P via symmetry.
 - out[t,v] = (Hs @ fc_wh.T + 1*fc_b) + diag(1/rowsum) @ (P @ G): the rowsum
   normalization lands on psum partitions (t) -> per-partition DVE scale.
"""

import os
import sys

sys.path.insert(0, "/opt/trn_rl_repo")

import numpy as np

import concourse.bass as bass
import concourse.bacc as bacc
import concourse.mybir as mybir
import concourse.tile as tile
from concourse.bass_utils import run_bass_kernel_spmd
from concourse.masks import make_identity

B, T, H, V = 128, 512, 1024, 128
NCORES = 8
BS = B // NCORES  # 16 batches per core
KCH = H // 128  # 8 hidden chunks
F32 = mybir.dt.float32
F32R = mybir.dt.float32r
BF16 = mybir.dt.bfloat16
AF = mybir.ActivationFunctionType
ALU = mybir.AluOpType

UNROLL = 8


def r32(ap):
    return ap.bitcast(F32R)


def build_nc(t_steps=T):
    nc = bacc.Bacc(None, target_bir_lowering=False)
    n_oh_chunks = (t_steps * BS + 511) // 512  # onehot build chunks of 512 cols

    # ---- DRAM I/O ----
    whh_d = nc.dram_tensor("whh", [H, H], F32, kind="ExternalInput")
    wxhp_d = nc.dram_tensor("wxhp", [V, H], F32, kind="ExternalInput")  # Wxh + bh
    xt_d = nc.dram_tensor("xt", [n_oh_chunks, 512], F32, kind="ExternalInput")
    fcwt_d = nc.dram_tensor("fcwt", [2 * H, V], F32, kind="ExternalInput")  # fc_w.T
    fcb_d = nc.dram_tensor("fcb", [1, V], F32, kind="ExternalInput")
    out_d = nc.dram_tensor("out", [BS, t_steps, V], F32, kind="ExternalOutput")
    oh_d = nc.dram_tensor("oh", [V, n_oh_chunks * 512 + 256], BF16)  # scratch

    with tile.TileContext(nc) as tc:
        with tc.tile_pool(name="persist", bufs=1) as pp:
            # persistent SBUF
            hst = pp.tile([128, KCH * BS * t_steps], BF16, tag="hst")
            fcwt_sb = pp.tile([128, 16 * V], BF16, tag="fcwt")
            fcb_row = pp.tile([1, V], F32, tag="fcb")
            id_sb = pp.tile([128, 128], BF16, tag="ident")
            iota_f = pp.tile([128, 1], F32, tag="iotaf")
            hta = pp.tile([128, KCH * 32], BF16, tag="hta")
            htb = pp.tile([128, KCH * 32], BF16, tag="htb")

            ones_f = pp.tile([1, 128], F32, tag="onesf")
            nc.gpsimd.memset(ones_f[:], 1.0)
            id_f = pp.tile([128, 128], F32, tag="identf")
            make_identity(nc, id_f[:])
            nc.vector.tensor_copy(id_sb[:], id_f[:])
            iota_i = pp.tile([128, 1], mybir.dt.int32, tag="iotai")
            nc.gpsimd.iota(iota_i[:], pattern=[[0, 1]], base=0, channel_multiplier=1)
            nc.vector.tensor_copy(iota_f[:], iota_i[:])
            zs_f = pp.tile([128, KCH * 32], F32, tag="zsf")
            nc.gpsimd.memset(zs_f[:], 0.0)  # h0 = 0 (+ zero pad cols stay 0)
            nc.vector.tensor_copy(hta[:], zs_f[:])
            nc.vector.tensor_copy(htb[:], zs_f[:])
            nc.gpsimd.dma_start(
                fcwt_sb.rearrange("p (c v) -> p c v", c=16)[:, :, :],
                fcwt_d.rearrange("(c p) v -> p c v", p=128)[:, :, :],
            )
            nc.gpsimd.dma_start(fcb_row[:], fcb_d[:])

            with tc.tile_pool(name="rconst", bufs=1) as rc:
                whh_sb = rc.tile([128, KCH * H], BF16, tag="whh")
                wxhp_sb = rc.tile([128, H], BF16, tag="wxhp")
                whh_raw = rc.tile([128, KCH * H], BF16, tag="whhraw")
                wxhp_raw = rc.tile([128, H], BF16, tag="wxhpraw")
                nc.gpsimd.dma_start(
                    whh_raw.rearrange("p (k h) -> p k h", k=KCH)[:, :, :],
                    whh_d.rearrange("(k p) h -> p k h", p=128)[:, :, :],
                )
                nc.gpsimd.dma_start(wxhp_raw[:], wxhp_d[:])
                nc.vector.tensor_copy(whh_sb[:], whh_raw[:])
                nc.vector.tensor_copy(wxhp_sb[:], wxhp_raw[:])

                # ---- build onehot(x) in DRAM, t-major columns (t*BS + b) ----
                with (
                    tc.tile_pool(name="ohb", bufs=3) as ohb,
                    tc.tile_pool(name="psb", bufs=2, space="PSUM") as psb,
                ):
                  for j in range(n_oh_chunks):
                      xraw = ohb.tile([1, 512], F32, tag="xraw")
                      nc.gpsimd.dma_start(xraw[:], xt_d[j : j + 1, :])
                      xrow = ohb.tile([1, 512], F32, tag="xrow")
                      nc.vector.tensor_copy(xrow[:], xraw[:])
                      psx = psb.tile([128, 512], F32, tag="psx")
                      nc.tensor.matmul(
                          psx[:], ones_f[:], xrow[:], start=True, stop=True
                      )
                      oh_sb = ohb.tile([128, 512], BF16, tag="ohsb")
                      nc.vector.tensor_scalar(
                          out=oh_sb[:],
                          in0=psx[:],
                          scalar1=iota_f[:],
                          scalar2=None,
                          op0=ALU.is_equal,
                      )
                      nc.sync.dma_start(oh_d[:, j * 512 : (j + 1) * 512], oh_sb[:])

                # ---- recurrence ----
                with (
                    tc.tile_pool(name="ohs", bufs=2 * UNROLL) as ohs,
                    tc.tile_pool(name="hgrp", bufs=2) as hg,
                    tc.tile_pool(name="psz", bufs=1, space="PSUM") as psz_p,
                    tc.tile_pool(name="pst", bufs=2, space="PSUM") as pst_p,
                    tc.tile_pool(name="pdmy", bufs=1, space="PSUM") as pdmy_p,
                ):
                    pdmy_m = pdmy_p.tile([1, 1], F32, tag="pdmym")

                    def pe_fence():
                        # dummy normal-mode matmul with no cross-engine deps;
                        # absorbs the PE mode-transition/structural self-wait
                        # so real matmuls keep one sync wait (S3_LW limit)
                        nc.tensor.matmul(
                            pdmy_m[:], id_f[0:1, 0:1], id_f[0:1, 0:1],
                            start=True, stop=True,
                        )

                    def pe_fence_t():
                        # transpose-mode fence (before real transposes)
                        nc.tensor.transpose(
                            pdmy_m[:], id_f[0:1, 0:1], id_f[0:1, 0:1]
                        )
                    def step(t_expr, parity, t_next_expr=None):
                        pe_fence()
                        # prefetch onehot for this step (and issue early)
                        oh_raw = ohs.tile([128, BS], BF16, tag="ohraw")
                        nc.sync.dma_start(oh_raw[:], oh_d[:, bass.ts(t_expr, BS)])
                        # bounce through DVE so PE matmuls carry a single
                        # (DVE) wait — S3_LW codegen allows only one sync wait
                        oh_t = ohs.tile([128, 32], BF16, tag="oht")
                        nc.gpsimd.memset(oh_t[:, BS:32], 0.0)
                        nc.vector.tensor_copy(oh_t[:, 0:BS], oh_raw[:])
                        ht_cur = hta if parity == 0 else htb
                        ht_new = htb if parity == 0 else hta
                        pszl = [
                            psz_p.tile([128, 256], F32, tag=f"psz{g}", name=f"pszt{g}")
                            for g in range(4)
                        ]
                        for k in range(-1, KCH):
                            for g in range(4):
                                if k < 0:  # vocab chunk first: starts the group
                                    lhsT = oh_t[:]
                                    rhs = wxhp_sb[:, 256 * g : 256 * g + 256]
                                else:
                                    lhsT = ht_cur[:, 32 * k : 32 * k + 32]
                                    rhs = whh_sb[:, k * H + 256 * g : k * H + 256 * g + 256]
                                nc.tensor.matmul(
                                    pszl[g][32 * g : 32 * g + 32, :],
                                    lhsT,
                                    rhs,
                                    start=(k == -1),
                                    stop=(k == KCH - 1),
                                    tile_position=(0, 32 * g),
                                )
                        h_grp = hg.tile([128, 256], BF16, tag="hgrp")
                        for g in range(4):
                            nc.scalar.activation(
                                h_grp[32 * g : 32 * g + 32, :],
                                pszl[g][32 * g : 32 * g + 32, :],
                                AF.Tanh,
                            )
                        pe_fence_t()
                        pst = pst_p.tile([128, 128], BF16, tag="pst")
                        for k in range(KCH):
                            g, half = k // 2, k % 2
                            nc.tensor.transpose(
                                pst[:, BS * k : BS * k + BS],
                                h_grp[32 * g : 32 * g + BS, 128 * half : 128 * half + 128],
                                id_sb[32 * g : 32 * g + BS, 32 * g : 32 * g + BS],
                                tile_position=(32 * g, 0),
                            )
                        # hT for next step (zero-pad cols untouched)
                        ht_v = ht_new.rearrange("p (k c) -> p k c", k=KCH)
                        pst_v = pst.rearrange("p (k c) -> p k c", k=KCH)
                        nc.vector.tensor_copy(ht_v[:, :, 0:BS], pst_v[:, :, :])
                        # append to HsT (bf16), layout [128, (k, b, t)]
                        hst_v = hst.rearrange(
                            "p (kb t) -> p kb t", t=t_steps
                        )
                        nc.vector.tensor_copy(
                            hst_v[:, :, bass.ts(t_expr, 1)],
                            pst.rearrange("p (kb one) -> p kb one", one=1)[:, :, :],
                        )

                    if t_steps <= 32:
                        for t in range(t_steps):
                            step(t, t % 2)
                    else:
                        assert t_steps % UNROLL == 0
                        with tc.For_i(
                            0, t_steps, UNROLL, hint_engines=(mybir.EngineType.PE,)
                        ) as iv:
                            for s in range(UNROLL):
                                step(iv + s, s % 2)

            # ---- attention + fc, per batch ----
            with (
                tc.tile_pool(name="attn", bufs=1) as ap_,
                tc.tile_pool(name="attn2", bufs=2) as ap2,
                tc.tile_pool(name="psS", bufs=2, space="PSUM") as psS_p,
                tc.tile_pool(name="psG", bufs=2, space="PSUM") as psG_p,
                tc.tile_pool(name="ps1", bufs=2, space="PSUM") as ps1_p,
                tc.tile_pool(name="ps2", bufs=2, space="PSUM") as ps2_p,
            ):
                hst_v = hst.rearrange("p (kb t) -> p kb t", t=t_steps)
                n_tc = t_steps // 128  # t-chunks of 128
                for b in range(BS):
                    def hs(k, sl):  # HsT tile for (k-chunk, slice of t)
                        return hst_v[:, k * BS + b, sl]

                    p_sb = ap_.tile([128, n_tc * t_steps], F32R, tag="p_sb")
                    rinv = ap_.tile([128, n_tc], F32, tag="rinv")
                    for c in range(n_tc):
                        psS = psS_p.tile([128, t_steps], F32, tag="psS")
                        for k in range(KCH):
                            nc.tensor.matmul(
                                psS[:],
                                hs(k, slice(128 * c, 128 * c + 128)),
                                hs(k, slice(0, t_steps)),
                                start=(k == 0),
                                stop=(k == KCH - 1),
                            )
                        rowsum = ap2.tile([128, 1], F32, tag="rowsum")
                        nc.scalar.activation(
                            p_sb[:, c * t_steps : (c + 1) * t_steps],
                            psS[:],
                            AF.Exp,
                            accum_out=rowsum[:],
                        )
                        nc.vector.reciprocal(rinv[:, c : c + 1], rowsum[:])
                    # G = Hs @ fc_w[:, H:].T  -> [t(=s) chunks, V]
                    g_sb = ap_.tile([128, n_tc * V], F32R, tag="g_sb")
                    for i in range(n_tc):
                        psG = psG_p.tile([128, V], F32, tag="psG")
                        for k in range(KCH):
                            nc.tensor.matmul(
                                psG[:],
                                hs(k, slice(128 * i, 128 * i + 128)),
                                fcwt_sb[:, (KCH + k) * V : (KCH + k + 1) * V],
                                start=(k == 0),
                                stop=(k == KCH - 1),
                            )
                        nc.vector.tensor_copy(g_sb[:, i * V : (i + 1) * V], psG[:])
                    # out[t-chunk c] = Hs@fc_wh.T + ones*fc_b + rinv*(P @ G)
                    for c in range(n_tc):
                        ps1 = ps1_p.tile([128, V], F32, tag="ps1")
                        for k in range(KCH):
                            nc.tensor.matmul(
                                ps1[:],
                                hs(k, slice(128 * c, 128 * c + 128)),
                                fcwt_sb[:, k * V : (k + 1) * V],
                                start=(k == 0),
                                stop=False,
                            )
                        nc.tensor.matmul(
                            ps1[:],
                            ones_f[:],
                            fcb_row[:],
                            start=False,
                            stop=True,
                        )
                        ps2 = ps2_p.tile([128, V], F32, tag="ps2")
                        for i in range(n_tc):
                            # lhsT = P^T block (i,c) == P block, by symmetry of exp(S)
                            nc.tensor.matmul(
                                ps2[:],
                                p_sb[:, i * t_steps + 128 * c : i * t_steps + 128 * c + 128],
                                g_sb[:, i * V : (i + 1) * V],
                                start=(i == 0),
                                stop=(i == n_tc - 1),
                            )
                        o2 = ap2.tile([128, V], F32, tag="o2")
                        nc.vector.tensor_scalar_mul(o2[:], ps2[:], rinv[:, c : c + 1])
                        oo = ap2.tile([128, V], F32, tag="oo")
                        nc.vector.tensor_add(oo[:], ps1[:], o2[:])
                        nc.sync.dma_start(out_d[b, 128 * c : 128 * c + 128, :], oo[:])

    nc.compile()
    return nc


def _prep_core_inputs(inputs, core, t_steps=T):
    x = np.asarray(inputs["x"])[core * BS : (core + 1) * BS, :t_steps]
    wxhp = (
        np.asarray(inputs["Wxh"]).astype(np.float32)
        + np.asarray(inputs["bh"]).astype(np.float32)[None, :]
    )
    n_oh_chunks = (t_steps * BS + 511) // 512
    xt = np.zeros(n_oh_chunks * 512, dtype=np.float32)
    xt[: t_steps * BS] = x.T.reshape(-1).astype(np.float32)  # col = t*BS + b
    return {
        "whh": np.ascontiguousarray(np.asarray(inputs["Whh"]).astype(np.float32)),
        "wxhp": np.ascontiguousarray(wxhp),
        "xt": xt.reshape(n_oh_chunks, 512),
        "fcwt": np.ascontiguousarray(
            np.asarray(inputs["fc_w"]).astype(np.float32).T
        ),
        "fcb": np.asarray(inputs["fc_b"]).astype(np.float32).reshape(1, V),
    }


def kernel(x, Wxh, Whh, bh, fc_w, fc_b, t_steps=T, trace=False):
    inputs = dict(x=x, Wxh=Wxh, Whh=Whh, bh=bh, fc_w=fc_w, fc_b=fc_b)
    nc = build_nc(t_steps)
    in_maps = [_prep_core_inputs(inputs, c, t_steps) for c in range(NCORES)]
    res = run_bass_kernel_spmd(nc, in_maps, core_ids=list(range(NCORES)), trace=trace)
    out = np.concatenate([r["out"] for r in res.results], axis=0)
    if trace:
        print(f"HW exec time: {res.exec_time_ns} ns", file=sys.stderr)
    return out



# revision 14
# speedup vs baseline: 1.3122x; 1.3122x over previous
"""AttentionRNN Trainium2 kernel — hybrid dual-port recurrence.

Problem: B=128, T=512, H=1024, V=128
  xe = Wxh[x]                               (gather == onehot(x) @ Wxh)
  h_t = tanh(xe_t + h_{t-1} @ Whh + bh)     (512 sequential steps)
  S   = Hs @ Hs^T  (per batch);  W = softmax(S, axis=-1)
  ctx = W @ Hs;    out = [Hs, ctx] @ fc_w.T + fc_b

Sharding: data-parallel over batch, 16 batches per core, 8 cores. Params
replicated. No collectives.

Recurrence design (the bottleneck): per step ~1.15 M Whh elements must
enter the PE array. The array has TWO independent SBUF read ports:
 - moving-operand port (~128 elem/cycle @ 2.4 GHz)
 - weight port (FWL, ~2 bf16 elem/cycle/partition @ 1.2 GHz)
A single formulation uses only one port. We split the output hidden dim:
 - part-a (j in [0,512)): h-stationary. lhsT = h chunk [128,16] (cheap
   16-col weight loads), Whh streams as the MOVING operand. 4 PE column
   groups (tile_position=(0,32g)), group g computes j-chunk g, N=128.
   psum [32g+b, c] = z[b, 128g+c] -> grouped layout, tanh -> ONE
   [128,128] PE transpose gives hidden-major h for chunks 0-3.
 - part-b (j in [512,1024)): Whh-stationary z^T form. lhsT = Whh chunk
   [128,128] (flows through the WEIGHT port via fast-weight-load,
   overlapped with part-a's streaming), rhs = h chunk [128,16], N=16.
   psum [p, 16jj+b] = z[b, 512+128jj+p] -> hidden-major DIRECTLY (no
   transpose), tanh writes ht chunks 4-7.
Both ports run concurrently -> ~2x the single-port floor. The onehot
vocab contribution rides in both parts (host-precomputed onehot in SBUF).
State ht ping-pongs [128, (k8 b16)] bf16 hidden-major; each step appends
h_t to HsT (bf16) for the attention phase off the critical path.

Attention (per batch): exp WITHOUT max-subtraction (P = exp(S) symmetric),
ctx@fc_wc.T = P @ (Hs@fc_wc.T) = P @ G associativity, P^T blocks read from
P via symmetry, rowsum normalization via per-partition DVE scale. All
matmul operands bf16 (fp32r pays 4 cycles/row at N<256).
"""

import os
import sys

sys.path.insert(0, "/opt/trn_rl_repo")

import ml_dtypes
import numpy as np

import concourse.bass as bass
import concourse.bacc as bacc
import concourse.mybir as mybir
import concourse.tile as tile
from concourse.bass_utils import run_bass_kernel_spmd
from concourse.masks import make_identity

B, T, H, V = 128, 512, 1024, 128
NCORES = 8
BS = B // NCORES  # 16 batches per core
KCH = H // 128  # 8 hidden chunks
F32 = mybir.dt.float32
BF16 = mybir.dt.bfloat16
AF = mybir.ActivationFunctionType
BFNP = ml_dtypes.bfloat16

UNROLL = 8


def build_nc(t_steps=T):
    nc = bacc.Bacc(None, target_bir_lowering=False)

    # ---- DRAM I/O (all weights host-prepped bf16) ----
    whh_a_d = nc.dram_tensor("whh_a", [128, KCH * 512], BF16, kind="ExternalInput")
    whh_b_d = nc.dram_tensor("whh_b", [128, 4 * KCH * 128], BF16, kind="ExternalInput")
    wxh_a_d = nc.dram_tensor("wxh_a", [V, 512], BF16, kind="ExternalInput")
    wxh_b_d = nc.dram_tensor("wxh_b", [V, 512], BF16, kind="ExternalInput")
    oh_d = nc.dram_tensor("oh", [V, (t_steps + UNROLL) * BS], BF16, kind="ExternalInput")
    fcwt_d = nc.dram_tensor("fcwt", [2 * H, V], BF16, kind="ExternalInput")
    fcb_d = nc.dram_tensor("fcb", [1, V], BF16, kind="ExternalInput")
    out_d = nc.dram_tensor("out", [BS, t_steps, V], F32, kind="ExternalOutput")
    hdbg_d = None
    if os.environ.get("HDBG", "0") == "1":
        hdbg_d = nc.dram_tensor(
            "hdbg", [128, KCH * BS * t_steps], BF16, kind="ExternalOutput"
        )

    with tile.TileContext(nc) as tc:
        with tc.tile_pool(name="persist", bufs=1) as pp:
            hst = pp.tile([128, KCH * BS * t_steps], BF16, tag="hst")
            whh_a = pp.tile([128, KCH * 512], BF16, tag="whh_a")
            whh_b = pp.tile([128, 4 * KCH * 128], BF16, tag="whh_b")
            wxh_a = pp.tile([128, 512], BF16, tag="wxh_a")
            wxh_b = pp.tile([128, 512], BF16, tag="wxh_b")
            oh = pp.tile([128, (t_steps + UNROLL) * BS], BF16, tag="oh")
            ohwin = pp.tile([128, UNROLL * BS], BF16, tag="ohwin")
            fcwt_sb = pp.tile([128, 16 * V], BF16, tag="fcwt")
            fcb_row = pp.tile([1, V], BF16, tag="fcb")
            id_bf = pp.tile([128, 128], BF16, tag="ident")
            hta = pp.tile([128, 128], BF16, tag="hta")
            htb = pp.tile([128, 128], BF16, tag="htb")
            hga = pp.tile([128, 128], BF16, tag="hga")
            hgb = pp.tile([128, 128], BF16, tag="hgb")
            ones_bf = pp.tile([1, 128], BF16, tag="onesb")

            nc.gpsimd.memset(ones_bf[:], 1.0)
            nc.gpsimd.memset(hta[:], 0.0)  # h_{-1} = 0
            nc.gpsimd.memset(htb[:], 0.0)
            nc.gpsimd.memset(hga[:], 0.0)  # gap rows (32g+16..32g+32) stay 0
            nc.gpsimd.memset(hgb[:], 0.0)

            nc.gpsimd.dma_start(whh_a[:], whh_a_d[:])
            nc.gpsimd.dma_start(whh_b[:], whh_b_d[:])
            nc.gpsimd.dma_start(wxh_a[:], wxh_a_d[:])
            nc.gpsimd.dma_start(wxh_b[:], wxh_b_d[:])
            nc.gpsimd.dma_start(oh[:], oh_d[:])
            nc.gpsimd.dma_start(
                fcwt_sb.rearrange("p (c v) -> p c v", c=16)[:, :, :],
                fcwt_d.rearrange("(c p) v -> p c v", p=128)[:, :, :],
            )
            nc.gpsimd.dma_start(fcb_row[:], fcb_d[:])

            with tc.tile_pool(name="idtmp", bufs=1) as it_:
                id_f = it_.tile([128, 128], F32, tag="identf")
                make_identity(nc, id_f[:])
                nc.vector.tensor_copy(id_bf[:], id_f[:])

            hst_v = hst.rearrange("p (kb t) -> p kb t", t=t_steps)
            oh_w = oh.rearrange("p (t b) -> p t b", b=BS)
            ohwin_w = ohwin.rearrange("p (s b) -> p s b", b=BS)

            # ---- recurrence ----
            with (
                tc.tile_pool(name="psa", bufs=2, space="PSUM") as pap,
                tc.tile_pool(name="psb", bufs=2, space="PSUM") as pbp,
                tc.tile_pool(name="pst", bufs=2, space="PSUM") as ptp,
                tc.tile_pool(name="pdmy", bufs=1, space="PSUM") as pdmy_p,
            ):
                pdmy_m = pdmy_p.tile([1, 1], F32, tag="pdmym")
                pdmy_t = pdmy_p.tile([1, 1], BF16, tag="pdmyt")

                def pe_fence():
                    # dummy normal-mode matmul; absorbs PE mode-transition /
                    # structural self-waits so real matmuls keep a single
                    # sync wait (S3_LW codegen limit)
                    nc.tensor.matmul(
                        pdmy_m[:], id_bf[0:1, 0:1], id_bf[0:1, 0:1],
                        start=True, stop=True,
                    )

                def pe_fence_t():
                    nc.tensor.transpose(
                        pdmy_t[:], id_bf[0:1, 0:1], id_bf[0:1, 0:1]
                    )

                def step(t_expr, parity, s=None, iv=None):
                    ht_cur = hta if parity == 0 else htb
                    ht_new = htb if parity == 0 else hta
                    h_grp = hga if parity == 0 else hgb
                    if s is None:
                        # python-unrolled path: fixed window position
                        oh_t = ohwin_w[:, t_expr % UNROLL, :]
                    else:
                        oh_t = ohwin_w[:, s, :]  # fixed address (LDWEIGHTS)

                    pe_fence()
                    pa = pap.tile([128, 128], F32, tag="pa", name="pa")
                    pb = pbp.tile([128, 64], F32, tag="pb", name="pb")
                    # vocab first: no dependency on previous step's h.
                    # start=True clears the whole PSUM bank, so only the
                    # first matmul into pb carries it; later column slices
                    # land on has_written=0 cells (overwrite mode).
                    for jj in range(4):
                        nc.tensor.matmul(
                            pb[:, 16 * jj : 16 * jj + 16],
                            wxh_b[:, 128 * jj : 128 * jj + 128],
                            oh_t,
                            start=(jj == 0), stop=False,
                            skip_group_check=True,
                        )
                    for g in range(4):
                        nc.tensor.matmul(
                            pa[32 * g : 32 * g + 16, :],
                            oh_t,
                            wxh_a[:, 128 * g : 128 * g + 128],
                            start=True, stop=False,
                            tile_position=(0, 32 * g),
                        )
                    if s == UNROLL - 1 and iv is not None:
                        # prefetch next onehot window (single buffer; WAR on
                        # this window's vocab matmuls above keeps semantics)
                        nc.vector.tensor_copy(
                            ohwin_w[:, :, :],
                            oh_w[:, bass.DynSlice(iv + UNROLL, UNROLL), :],
                        )
                    # hidden: chunks 4-7 first (ready earliest: direct ACT of
                    # previous step), then 0-3 (via transpose + DVE copy)
                    for k in (4, 5, 6, 7, 0, 1, 2, 3):
                        last = k == 3
                        hk = ht_cur[:, 16 * k : 16 * k + 16]
                        for jj in range(4):
                            nc.tensor.matmul(
                                pb[:, 16 * jj : 16 * jj + 16],
                                whh_b[:, (jj * KCH + k) * 128 : (jj * KCH + k) * 128 + 128],
                                hk,
                                start=False, stop=last,
                            )
                        for g in range(4):
                            nc.tensor.matmul(
                                pa[32 * g : 32 * g + 16, :],
                                hk,
                                whh_a[:, k * 512 + 128 * g : k * 512 + 128 * g + 128],
                                start=False, stop=last,
                                tile_position=(0, 32 * g),
                            )
                    # tanh: part-b lands hidden-major directly in ht chunks 4-7
                    nc.scalar.activation(ht_new[:, 64:128], pb[:, :], AF.Tanh)
                    for g in range(4):
                        nc.scalar.activation(
                            h_grp[32 * g : 32 * g + 16, :],
                            pa[32 * g : 32 * g + 16, :],
                            AF.Tanh,
                        )
                    # part-a: one [128,128] transpose -> hidden-major chunks 0-3
                    pe_fence_t()
                    pt = ptp.tile([128, 128], BF16, tag="pt", name="pt")
                    nc.tensor.transpose(pt[:, :], h_grp[:, :], id_bf[:, :])
                    pt_v = pt.rearrange("p (g c) -> p g c", g=4)
                    htn_v = ht_new.rearrange("p (k b) -> p k b", k=KCH)
                    nc.vector.tensor_copy(htn_v[:, 0:4, :], pt_v[:, :, 0:BS])
                    # append h_t to HsT (off critical path)
                    nc.vector.tensor_copy(
                        hst_v[:, :, bass.ts(t_expr, 1)],
                        ht_new.rearrange("p (kb one) -> p kb one", one=1)[:, :, :],
                    )

                if t_steps <= 32:
                    for t in range(t_steps):
                        if t % UNROLL == 0:
                            nc.vector.tensor_copy(
                                ohwin_w[:, :, :],
                                oh_w[:, t : t + UNROLL, :],
                            )
                        step(t, t % 2)
                else:
                    assert t_steps % UNROLL == 0
                    nc.vector.tensor_copy(
                        ohwin_w[:, :, :], oh_w[:, 0:UNROLL, :]
                    )
                    with tc.For_i(
                        0, t_steps, UNROLL, hint_engines=(mybir.EngineType.PE,)
                    ) as iv:
                        for s in range(UNROLL):
                            step(iv + s, s % 2, s=s, iv=iv)

            if hdbg_d is not None:
                nc.sync.dma_start(hdbg_d[:, :], hst[:, :])

            # ---- attention + fc, per batch ----
            with (
                tc.tile_pool(name="attn", bufs=1) as ap_,
                tc.tile_pool(name="attn2", bufs=2) as ap2,
                tc.tile_pool(name="psS", bufs=2, space="PSUM") as psS_p,
                tc.tile_pool(name="psG", bufs=2, space="PSUM") as psG_p,
                tc.tile_pool(name="ps1", bufs=2, space="PSUM") as ps1_p,
                tc.tile_pool(name="ps2", bufs=2, space="PSUM") as ps2_p,
            ):
                n_tc = t_steps // 128  # t-chunks of 128
                for b in range(BS):
                    def hs(k, sl):  # HsT tile for (k-chunk, slice of t)
                        return hst_v[:, k * BS + b, sl]

                    p_sb = ap_.tile([128, n_tc * t_steps], BF16, tag="p_sb")
                    rinv = ap_.tile([128, n_tc], F32, tag="rinv")
                    for c in range(n_tc):
                        psS = psS_p.tile([128, t_steps], F32, tag="psS")
                        for k in range(KCH):
                            nc.tensor.matmul(
                                psS[:],
                                hs(k, slice(128 * c, 128 * c + 128)),
                                hs(k, slice(0, t_steps)),
                                start=(k == 0),
                                stop=(k == KCH - 1),
                            )
                        rowsum = ap2.tile([128, 1], F32, tag="rowsum")
                        nc.scalar.activation(
                            p_sb[:, c * t_steps : (c + 1) * t_steps],
                            psS[:],
                            AF.Exp,
                            accum_out=rowsum[:],
                        )
                        nc.vector.reciprocal(rinv[:, c : c + 1], rowsum[:])
                    # G = Hs @ fc_w[:, H:].T  -> [t(=s) chunks, V]
                    g_sb = ap_.tile([128, n_tc * V], BF16, tag="g_sb")
                    for i in range(n_tc):
                        psG = psG_p.tile([128, V], F32, tag="psG")
                        for k in range(KCH):
                            nc.tensor.matmul(
                                psG[:],
                                hs(k, slice(128 * i, 128 * i + 128)),
                                fcwt_sb[:, (KCH + k) * V : (KCH + k + 1) * V],
                                start=(k == 0),
                                stop=(k == KCH - 1),
                            )
                        nc.vector.tensor_copy(g_sb[:, i * V : (i + 1) * V], psG[:])
                    # out[t-chunk c] = Hs@fc_wh.T + ones*fc_b + rinv*(P @ G)
                    for c in range(n_tc):
                        ps1 = ps1_p.tile([128, V], F32, tag="ps1")
                        for k in range(KCH):
                            nc.tensor.matmul(
                                ps1[:],
                                hs(k, slice(128 * c, 128 * c + 128)),
                                fcwt_sb[:, k * V : (k + 1) * V],
                                start=(k == 0),
                                stop=False,
                            )
                        nc.tensor.matmul(
                            ps1[:],
                            ones_bf[:],
                            fcb_row[:],
                            start=False,
                            stop=True,
                        )
                        ps2 = ps2_p.tile([128, V], F32, tag="ps2")
                        for i in range(n_tc):
                            # lhsT = P^T block (i,c) == P block, by symmetry
                            nc.tensor.matmul(
                                ps2[:],
                                p_sb[:, i * t_steps + 128 * c : i * t_steps + 128 * c + 128],
                                g_sb[:, i * V : (i + 1) * V],
                                start=(i == 0),
                                stop=(i == n_tc - 1),
                            )
                        o2 = ap2.tile([128, V], F32, tag="o2")
                        nc.vector.tensor_scalar_mul(o2[:], ps2[:], rinv[:, c : c + 1])
                        oo = ap2.tile([128, V], F32, tag="oo")
                        nc.vector.tensor_add(oo[:], ps1[:], o2[:])
                        nc.sync.dma_start(out_d[b, 128 * c : 128 * c + 128, :], oo[:])

    nc.compile()
    return nc


def _prep_core_inputs(inputs, core, t_steps=T):
    x = np.asarray(inputs["x"])[core * BS : (core + 1) * BS, :t_steps]
    wxhb = (
        np.asarray(inputs["Wxh"]).astype(np.float32)
        + np.asarray(inputs["bh"]).astype(np.float32)[None, :]
    )
    whh = np.asarray(inputs["Whh"]).astype(np.float32)
    w4 = whh.reshape(KCH, 128, KCH, 128)
    # whh_a[p, (k, g, j)] = Whh[128k+p, 128g+j], g < 4
    whh_a = np.ascontiguousarray(w4[:, :, :4, :].transpose(1, 0, 2, 3)).reshape(128, -1)
    # whh_b[p, (jj, k, j)] = Whh[128k+p, 512 + 128jj + j]
    whh_b = np.ascontiguousarray(w4[:, :, 4:, :].transpose(1, 2, 0, 3)).reshape(128, -1)
    # onehot: oh[v, 16t + b] = (x[b, t] == v); one zero window of padding
    oh = np.zeros((V, (t_steps + UNROLL) * BS), dtype=BFNP)
    oh[x.T.reshape(-1), np.arange(t_steps * BS)] = 1
    return {
        "whh_a": whh_a.astype(BFNP),
        "whh_b": whh_b.astype(BFNP),
        "wxh_a": np.ascontiguousarray(wxhb[:, :512]).astype(BFNP),
        "wxh_b": np.ascontiguousarray(wxhb[:, 512:]).astype(BFNP),
        "oh": oh,
        "fcwt": np.ascontiguousarray(
            np.asarray(inputs["fc_w"]).astype(np.float32).T
        ).astype(BFNP),
        "fcb": np.asarray(inputs["fc_b"]).astype(np.float32).reshape(1, V).astype(BFNP),
    }


def kernel(x, Wxh, Whh, bh, fc_w, fc_b, t_steps=T, trace=False):
    inputs = dict(x=x, Wxh=Wxh, Whh=Whh, bh=bh, fc_w=fc_w, fc_b=fc_b)
    nc = build_nc(t_steps)
    in_maps = [_prep_core_inputs(inputs, c, t_steps) for c in range(NCORES)]
    res = run_bass_kernel_spmd(nc, in_maps, core_ids=list(range(NCORES)), trace=trace)
    out = np.concatenate([r["out"] for r in res.results], axis=0)
    if trace:
        print(f"HW exec time: {res.exec_time_ns} ns", file=sys.stderr)
    return out


# revision 15
# speedup vs baseline: 1.3214x; 1.0070x over previous
"""AttentionRNN Trainium2 kernel — hybrid dual-port recurrence.

Problem: B=128, T=512, H=1024, V=128
  xe = Wxh[x]                               (gather == onehot(x) @ Wxh)
  h_t = tanh(xe_t + h_{t-1} @ Whh + bh)     (512 sequential steps)
  S   = Hs @ Hs^T  (per batch);  W = softmax(S, axis=-1)
  ctx = W @ Hs;    out = [Hs, ctx] @ fc_w.T + fc_b

Sharding: data-parallel over batch, 16 batches per core, 8 cores. Params
replicated. No collectives.

Recurrence design (the bottleneck): per step ~1.15 M Whh elements must
enter the PE array. The array has TWO independent SBUF read ports:
 - moving-operand port (~128 elem/cycle @ 2.4 GHz)
 - weight port (FWL, ~2 bf16 elem/cycle/partition @ 1.2 GHz)
A single formulation uses only one port. We split the output hidden dim:
 - part-a (j in [0,512)): h-stationary. lhsT = h chunk [128,16] (cheap
   16-col weight loads), Whh streams as the MOVING operand. 4 PE column
   groups (tile_position=(0,32g)), group g computes j-chunk g, N=128.
   psum [32g+b, c] = z[b, 128g+c] -> grouped layout, tanh -> ONE
   [128,128] PE transpose gives hidden-major h for chunks 0-3.
 - part-b (j in [512,1024)): Whh-stationary z^T form. lhsT = Whh chunk
   [128,128] (flows through the WEIGHT port via fast-weight-load,
   overlapped with part-a's streaming), rhs = h chunk [128,16], N=16.
   psum [p, 16jj+b] = z[b, 512+128jj+p] -> hidden-major DIRECTLY (no
   transpose), tanh writes ht chunks 4-7.
Both ports run concurrently -> ~2x the single-port floor. The onehot
vocab contribution rides in both parts (host-precomputed onehot in SBUF).
State ht ping-pongs [128, (k8 b16)] bf16 hidden-major; each step appends
h_t to HsT (bf16) for the attention phase off the critical path.

Attention (per batch): exp WITHOUT max-subtraction (P = exp(S) symmetric),
ctx@fc_wc.T = P @ (Hs@fc_wc.T) = P @ G associativity, P^T blocks read from
P via symmetry, rowsum normalization via per-partition DVE scale. All
matmul operands bf16 (fp32r pays 4 cycles/row at N<256).
"""

import os
import sys

sys.path.insert(0, "/opt/trn_rl_repo")

import ml_dtypes
import numpy as np

import concourse.bass as bass
import concourse.bacc as bacc
import concourse.mybir as mybir
import concourse.tile as tile
from concourse.bass_utils import run_bass_kernel_spmd
from concourse.masks import make_identity

B, T, H, V = 128, 512, 1024, 128
NCORES = 8
BS = B // NCORES  # 16 batches per core
KCH = H // 128  # 8 hidden chunks
F32 = mybir.dt.float32
BF16 = mybir.dt.bfloat16
AF = mybir.ActivationFunctionType
BFNP = ml_dtypes.bfloat16

UNROLL = 8


def build_nc(t_steps=T):
    nc = bacc.Bacc(None, target_bir_lowering=False)

    # ---- DRAM I/O (all weights host-prepped bf16) ----
    whh_a_d = nc.dram_tensor("whh_a", [128, KCH * 512], BF16, kind="ExternalInput")
    whh_b_d = nc.dram_tensor("whh_b", [128, 4 * KCH * 128], BF16, kind="ExternalInput")
    wxh_a_d = nc.dram_tensor("wxh_a", [V, 512], BF16, kind="ExternalInput")
    wxh_b_d = nc.dram_tensor("wxh_b", [V, 512], BF16, kind="ExternalInput")
    oh_d = nc.dram_tensor("oh", [V, (t_steps + UNROLL) * BS], BF16, kind="ExternalInput")
    fcwt_d = nc.dram_tensor("fcwt", [2 * H, V], BF16, kind="ExternalInput")
    fcb_d = nc.dram_tensor("fcb", [1, V], BF16, kind="ExternalInput")
    out_d = nc.dram_tensor("out", [BS, t_steps, V], F32, kind="ExternalOutput")
    hdbg_d = None
    if os.environ.get("HDBG", "0") == "1":
        hdbg_d = nc.dram_tensor(
            "hdbg", [128, KCH * BS * t_steps], BF16, kind="ExternalOutput"
        )

    with tile.TileContext(nc) as tc:
        with tc.tile_pool(name="persist", bufs=1) as pp:
            hst = pp.tile([128, KCH * BS * t_steps], BF16, tag="hst")
            whh_a = pp.tile([128, KCH * 512], BF16, tag="whh_a")
            whh_b = pp.tile([128, 4 * KCH * 128], BF16, tag="whh_b")
            wxh_a = pp.tile([128, 512], BF16, tag="wxh_a")
            wxh_b = pp.tile([128, 512], BF16, tag="wxh_b")
            oh = pp.tile([128, (t_steps + UNROLL) * BS], BF16, tag="oh")
            ohwin = pp.tile([128, UNROLL * BS], BF16, tag="ohwin")
            fcwt_sb = pp.tile([128, 16 * V], BF16, tag="fcwt")
            fcb_row = pp.tile([1, V], BF16, tag="fcb")
            id_bf = pp.tile([128, 128], BF16, tag="ident")
            hta = pp.tile([128, 128], BF16, tag="hta")
            htb = pp.tile([128, 128], BF16, tag="htb")
            hga = pp.tile([128, 128], BF16, tag="hga")
            hgb = pp.tile([128, 128], BF16, tag="hgb")
            ones_bf = pp.tile([1, 128], BF16, tag="onesb")

            nc.gpsimd.memset(ones_bf[:], 1.0)
            nc.gpsimd.memset(hta[:], 0.0)  # h_{-1} = 0
            nc.gpsimd.memset(htb[:], 0.0)
            nc.gpsimd.memset(hga[:], 0.0)  # gap rows (32g+16..32g+32) stay 0
            nc.gpsimd.memset(hgb[:], 0.0)

            nc.gpsimd.dma_start(whh_a[:], whh_a_d[:])
            nc.gpsimd.dma_start(whh_b[:], whh_b_d[:])
            nc.gpsimd.dma_start(wxh_a[:], wxh_a_d[:])
            nc.gpsimd.dma_start(wxh_b[:], wxh_b_d[:])
            nc.gpsimd.dma_start(oh[:], oh_d[:])
            nc.gpsimd.dma_start(
                fcwt_sb.rearrange("p (c v) -> p c v", c=16)[:, :, :],
                fcwt_d.rearrange("(c p) v -> p c v", p=128)[:, :, :],
            )
            nc.gpsimd.dma_start(fcb_row[:], fcb_d[:])

            with tc.tile_pool(name="idtmp", bufs=1) as it_:
                id_f = it_.tile([128, 128], F32, tag="identf")
                make_identity(nc, id_f[:])
                nc.vector.tensor_copy(id_bf[:], id_f[:])

            hst_v = hst.rearrange("p (kb t) -> p kb t", t=t_steps)
            oh_w = oh.rearrange("p (t b) -> p t b", b=BS)
            ohwin_w = ohwin.rearrange("p (s b) -> p s b", b=BS)

            # ---- recurrence ----
            with (
                tc.tile_pool(name="psa", bufs=2, space="PSUM") as pap,
                tc.tile_pool(name="psb", bufs=2, space="PSUM") as pbp,
                tc.tile_pool(name="pst", bufs=2, space="PSUM") as ptp,
                tc.tile_pool(name="pdmy", bufs=1, space="PSUM") as pdmy_p,
            ):
                pdmy_m = pdmy_p.tile([1, 1], F32, tag="pdmym")
                pdmy_t = pdmy_p.tile([1, 1], BF16, tag="pdmyt")

                def pe_fence():
                    # dummy normal-mode matmul; absorbs PE mode-transition /
                    # structural self-waits so real matmuls keep a single
                    # sync wait (S3_LW codegen limit)
                    nc.tensor.matmul(
                        pdmy_m[:], id_bf[0:1, 0:1], id_bf[0:1, 0:1],
                        start=True, stop=True,
                    )

                def pe_fence_t():
                    nc.tensor.transpose(
                        pdmy_t[:], id_bf[0:1, 0:1], id_bf[0:1, 0:1]
                    )

                def step(t_expr, parity, s=None, iv=None):
                    ht_cur = hta if parity == 0 else htb
                    ht_new = htb if parity == 0 else hta
                    h_grp = hga if parity == 0 else hgb
                    if s is None:
                        # python-unrolled path: fixed window position
                        oh_t = ohwin_w[:, t_expr % UNROLL, :]
                    else:
                        oh_t = ohwin_w[:, s, :]  # fixed address (LDWEIGHTS)

                    pe_fence()
                    pa = pap.tile([128, 128], F32, tag="pa", name="pa")
                    pb = pbp.tile([128, 64], F32, tag="pb", name="pb")
                    # vocab first: no dependency on previous step's h.
                    # start=True clears the whole PSUM bank, so only the
                    # first matmul into pb carries it; later column slices
                    # land on has_written=0 cells (overwrite mode).
                    for jj in range(4):
                        nc.tensor.matmul(
                            pb[:, 16 * jj : 16 * jj + 16],
                            wxh_b[:, 128 * jj : 128 * jj + 128],
                            oh_t,
                            start=(jj == 0), stop=False,
                            skip_group_check=True,
                        )
                    for g in range(4):
                        nc.tensor.matmul(
                            pa[32 * g : 32 * g + 16, :],
                            oh_t,
                            wxh_a[:, 128 * g : 128 * g + 128],
                            start=True, stop=False,
                            tile_position=(0, 32 * g),
                        )
                    if s == UNROLL - 1 and iv is not None:
                        # prefetch next onehot window (single buffer; WAR on
                        # this window's vocab matmuls above keeps semantics)
                        nc.vector.tensor_copy(
                            ohwin_w[:, :, :],
                            oh_w[:, bass.DynSlice(iv + UNROLL, UNROLL), :],
                        )
                    # hidden: chunks 4-7 first (ready earliest: direct ACT of
                    # previous step), then 0-3 (via transpose + DVE copy)
                    for k in (4, 5, 6, 7, 0, 1, 2, 3):
                        last = k == 3
                        hk = ht_cur[:, 16 * k : 16 * k + 16]
                        for jj in range(4):
                            nc.tensor.matmul(
                                pb[:, 16 * jj : 16 * jj + 16],
                                whh_b[:, (jj * KCH + k) * 128 : (jj * KCH + k) * 128 + 128],
                                hk,
                                start=False, stop=last,
                            )
                        for g in range(4):
                            nc.tensor.matmul(
                                pa[32 * g : 32 * g + 16, :],
                                hk,
                                whh_a[:, k * 512 + 128 * g : k * 512 + 128 * g + 128],
                                start=False, stop=last,
                                tile_position=(0, 32 * g),
                            )
                    # tanh: part-b lands hidden-major directly in ht chunks 4-7
                    nc.scalar.activation(ht_new[:, 64:128], pb[:, :], AF.Tanh)
                    for g in range(4):
                        nc.scalar.activation(
                            h_grp[32 * g : 32 * g + 16, :],
                            pa[32 * g : 32 * g + 16, :],
                            AF.Tanh,
                        )
                    # part-a: one [128,128] transpose -> hidden-major chunks 0-3
                    pe_fence_t()
                    pt = ptp.tile([128, 128], BF16, tag="pt", name="pt")
                    nc.tensor.transpose(pt[:, :], h_grp[:, :], id_bf[:, :])
                    pt_v = pt.rearrange("p (g c) -> p g c", g=4)
                    htn_v = ht_new.rearrange("p (k b) -> p k b", k=KCH)
                    nc.vector.tensor_copy(htn_v[:, 0:4, :], pt_v[:, :, 0:BS])
                    # append h_t to HsT (off critical path)
                    nc.vector.tensor_copy(
                        hst_v[:, :, bass.ts(t_expr, 1)],
                        ht_new.rearrange("p (kb one) -> p kb one", one=1)[:, :, :],
                    )

                if t_steps <= 32:
                    for t in range(t_steps):
                        if t % UNROLL == 0:
                            nc.vector.tensor_copy(
                                ohwin_w[:, :, :],
                                oh_w[:, t : t + UNROLL, :],
                            )
                        step(t, t % 2)
                else:
                    assert t_steps % UNROLL == 0
                    nc.vector.tensor_copy(
                        ohwin_w[:, :, :], oh_w[:, 0:UNROLL, :]
                    )
                    with tc.For_i(
                        0, t_steps, UNROLL,
                        hint_engines=(mybir.EngineType.PE,),
                        staggered_reset=True,
                    ) as iv:
                        for s in range(UNROLL):
                            step(iv + s, s % 2, s=s, iv=iv)

            if hdbg_d is not None:
                nc.sync.dma_start(hdbg_d[:, :], hst[:, :])

            # ---- attention + fc, per batch ----
            with (
                tc.tile_pool(name="attn", bufs=1) as ap_,
                tc.tile_pool(name="attn2", bufs=2) as ap2,
                tc.tile_pool(name="psS", bufs=2, space="PSUM") as psS_p,
                tc.tile_pool(name="psG", bufs=2, space="PSUM") as psG_p,
                tc.tile_pool(name="ps1", bufs=2, space="PSUM") as ps1_p,
                tc.tile_pool(name="ps2", bufs=2, space="PSUM") as ps2_p,
            ):
                n_tc = t_steps // 128  # t-chunks of 128
                for b in range(BS):
                    def hs(k, sl):  # HsT tile for (k-chunk, slice of t)
                        return hst_v[:, k * BS + b, sl]

                    p_sb = ap_.tile([128, n_tc * t_steps], BF16, tag="p_sb")
                    rinv = ap_.tile([128, n_tc], F32, tag="rinv")
                    for c in range(n_tc):
                        psS = psS_p.tile([128, t_steps], F32, tag="psS")
                        for k in range(KCH):
                            nc.tensor.matmul(
                                psS[:],
                                hs(k, slice(128 * c, 128 * c + 128)),
                                hs(k, slice(0, t_steps)),
                                start=(k == 0),
                                stop=(k == KCH - 1),
                            )
                        rowsum = ap2.tile([128, 1], F32, tag="rowsum")
                        nc.scalar.activation(
                            p_sb[:, c * t_steps : (c + 1) * t_steps],
                            psS[:],
                            AF.Exp,
                            accum_out=rowsum[:],
                        )
                        nc.vector.reciprocal(rinv[:, c : c + 1], rowsum[:])
                    # G = Hs @ fc_w[:, H:].T  -> [t(=s) chunks, V]
                    g_sb = ap_.tile([128, n_tc * V], BF16, tag="g_sb")
                    for i in range(n_tc):
                        psG = psG_p.tile([128, V], F32, tag="psG")
                        for k in range(KCH):
                            nc.tensor.matmul(
                                psG[:],
                                hs(k, slice(128 * i, 128 * i + 128)),
                                fcwt_sb[:, (KCH + k) * V : (KCH + k + 1) * V],
                                start=(k == 0),
                                stop=(k == KCH - 1),
                            )
                        nc.vector.tensor_copy(g_sb[:, i * V : (i + 1) * V], psG[:])
                    # out[t-chunk c] = Hs@fc_wh.T + ones*fc_b + rinv*(P @ G)
                    for c in range(n_tc):
                        ps1 = ps1_p.tile([128, V], F32, tag="ps1")
                        for k in range(KCH):
                            nc.tensor.matmul(
                                ps1[:],
                                hs(k, slice(128 * c, 128 * c + 128)),
                                fcwt_sb[:, k * V : (k + 1) * V],
                                start=(k == 0),
                                stop=False,
                            )
                        nc.tensor.matmul(
                            ps1[:],
                            ones_bf[:],
                            fcb_row[:],
                            start=False,
                            stop=True,
                        )
                        ps2 = ps2_p.tile([128, V], F32, tag="ps2")
                        for i in range(n_tc):
                            # lhsT = P^T block (i,c) == P block, by symmetry
                            nc.tensor.matmul(
                                ps2[:],
                                p_sb[:, i * t_steps + 128 * c : i * t_steps + 128 * c + 128],
                                g_sb[:, i * V : (i + 1) * V],
                                start=(i == 0),
                                stop=(i == n_tc - 1),
                            )
                        o2 = ap2.tile([128, V], F32, tag="o2")
                        nc.vector.tensor_scalar_mul(o2[:], ps2[:], rinv[:, c : c + 1])
                        oo = ap2.tile([128, V], F32, tag="oo")
                        nc.vector.tensor_add(oo[:], ps1[:], o2[:])
                        nc.sync.dma_start(out_d[b, 128 * c : 128 * c + 128, :], oo[:])

    nc.compile()
    return nc


def _prep_core_inputs(inputs, core, t_steps=T):
    x = np.asarray(inputs["x"])[core * BS : (core + 1) * BS, :t_steps]
    wxhb = (
        np.asarray(inputs["Wxh"]).astype(np.float32)
        + np.asarray(inputs["bh"]).astype(np.float32)[None, :]
    )
    whh = np.asarray(inputs["Whh"]).astype(np.float32)
    w4 = whh.reshape(KCH, 128, KCH, 128)
    # whh_a[p, (k, g, j)] = Whh[128k+p, 128g+j], g < 4
    whh_a = np.ascontiguousarray(w4[:, :, :4, :].transpose(1, 0, 2, 3)).reshape(128, -1)
    # whh_b[p, (jj, k, j)] = Whh[128k+p, 512 + 128jj + j]
    whh_b = np.ascontiguousarray(w4[:, :, 4:, :].transpose(1, 2, 0, 3)).reshape(128, -1)
    # onehot: oh[v, 16t + b] = (x[b, t] == v); one zero window of padding
    oh = np.zeros((V, (t_steps + UNROLL) * BS), dtype=BFNP)
    oh[x.T.reshape(-1), np.arange(t_steps * BS)] = 1
    return {
        "whh_a": whh_a.astype(BFNP),
        "whh_b": whh_b.astype(BFNP),
        "wxh_a": np.ascontiguousarray(wxhb[:, :512]).astype(BFNP),
        "wxh_b": np.ascontiguousarray(wxhb[:, 512:]).astype(BFNP),
        "oh": oh,
        "fcwt": np.ascontiguousarray(
            np.asarray(inputs["fc_w"]).astype(np.float32).T
        ).astype(BFNP),
        "fcb": np.asarray(inputs["fc_b"]).astype(np.float32).reshape(1, V).astype(BFNP),
    }


def kernel(x, Wxh, Whh, bh, fc_w, fc_b, t_steps=T, trace=False):
    inputs = dict(x=x, Wxh=Wxh, Whh=Whh, bh=bh, fc_w=fc_w, fc_b=fc_b)
    nc = build_nc(t_steps)
    in_maps = [_prep_core_inputs(inputs, c, t_steps) for c in range(NCORES)]
    res = run_bass_kernel_spmd(nc, in_maps, core_ids=list(range(NCORES)), trace=trace)
    out = np.concatenate([r["out"] for r in res.results], axis=0)
    if trace:
        print(f"HW exec time: {res.exec_time_ns} ns", file=sys.stderr)
    return out


# revision 16
# speedup vs baseline: 1.3240x; 1.0020x over previous
"""AttentionRNN Trainium2 kernel — hybrid dual-port recurrence.

Problem: B=128, T=512, H=1024, V=128
  xe = Wxh[x]                               (gather == onehot(x) @ Wxh)
  h_t = tanh(xe_t + h_{t-1} @ Whh + bh)     (512 sequential steps)
  S   = Hs @ Hs^T  (per batch);  W = softmax(S, axis=-1)
  ctx = W @ Hs;    out = [Hs, ctx] @ fc_w.T + fc_b

Sharding: data-parallel over batch, 16 batches per core, 8 cores. Params
replicated. No collectives.

Recurrence design (the bottleneck): per step ~1.15 M Whh elements must
enter the PE array. The array has TWO independent SBUF read ports:
 - moving-operand port (~128 elem/cycle @ 2.4 GHz)
 - weight port (FWL, ~2 bf16 elem/cycle/partition @ 1.2 GHz)
A single formulation uses only one port. We split the output hidden dim:
 - part-a (j in [0,512)): h-stationary. lhsT = h chunk [128,16] (cheap
   16-col weight loads), Whh streams as the MOVING operand. 4 PE column
   groups (tile_position=(0,32g)), group g computes j-chunk g, N=128.
   psum [32g+b, c] = z[b, 128g+c] -> grouped layout, tanh -> ONE
   [128,128] PE transpose gives hidden-major h for chunks 0-3.
 - part-b (j in [512,1024)): Whh-stationary z^T form. lhsT = Whh chunk
   [128,128] (flows through the WEIGHT port via fast-weight-load,
   overlapped with part-a's streaming), rhs = h chunk [128,16], N=16.
   psum [p, 16jj+b] = z[b, 512+128jj+p] -> hidden-major DIRECTLY (no
   transpose), tanh writes ht chunks 4-7.
Both ports run concurrently -> ~2x the single-port floor. The onehot
vocab contribution rides in both parts (host-precomputed onehot in SBUF).
State ht ping-pongs [128, (k8 b16)] bf16 hidden-major; each step appends
h_t to HsT (bf16) for the attention phase off the critical path.

Attention (per batch): exp WITHOUT max-subtraction (P = exp(S) symmetric),
ctx@fc_wc.T = P @ (Hs@fc_wc.T) = P @ G associativity, P^T blocks read from
P via symmetry, rowsum normalization via per-partition DVE scale. All
matmul operands bf16 (fp32r pays 4 cycles/row at N<256).
"""

import os
import sys

sys.path.insert(0, "/opt/trn_rl_repo")

import ml_dtypes
import numpy as np

import concourse.bass as bass
import concourse.bacc as bacc
import concourse.mybir as mybir
import concourse.tile as tile
from concourse.bass_utils import run_bass_kernel_spmd
from concourse.masks import make_identity

B, T, H, V = 128, 512, 1024, 128
NCORES = 8
BS = B // NCORES  # 16 batches per core
KCH = H // 128  # 8 hidden chunks
F32 = mybir.dt.float32
BF16 = mybir.dt.bfloat16
AF = mybir.ActivationFunctionType
BFNP = ml_dtypes.bfloat16

UNROLL = 16


def build_nc(t_steps=T):
    nc = bacc.Bacc(None, target_bir_lowering=False)

    # ---- DRAM I/O (all weights host-prepped bf16) ----
    whh_a_d = nc.dram_tensor("whh_a", [128, KCH * 512], BF16, kind="ExternalInput")
    whh_b_d = nc.dram_tensor("whh_b", [128, 4 * KCH * 128], BF16, kind="ExternalInput")
    wxh_a_d = nc.dram_tensor("wxh_a", [V, 512], BF16, kind="ExternalInput")
    wxh_b_d = nc.dram_tensor("wxh_b", [V, 512], BF16, kind="ExternalInput")
    oh_d = nc.dram_tensor("oh", [V, (t_steps + UNROLL) * BS], BF16, kind="ExternalInput")
    fcwt_d = nc.dram_tensor("fcwt", [2 * H, V], BF16, kind="ExternalInput")
    fcb_d = nc.dram_tensor("fcb", [1, V], BF16, kind="ExternalInput")
    out_d = nc.dram_tensor("out", [BS, t_steps, V], F32, kind="ExternalOutput")
    hdbg_d = None
    if os.environ.get("HDBG", "0") == "1":
        hdbg_d = nc.dram_tensor(
            "hdbg", [128, KCH * BS * t_steps], BF16, kind="ExternalOutput"
        )

    with tile.TileContext(nc) as tc:
        with tc.tile_pool(name="persist", bufs=1) as pp:
            hst = pp.tile([128, KCH * BS * t_steps], BF16, tag="hst")
            whh_a = pp.tile([128, KCH * 512], BF16, tag="whh_a")
            whh_b = pp.tile([128, 4 * KCH * 128], BF16, tag="whh_b")
            wxh_a = pp.tile([128, 512], BF16, tag="wxh_a")
            wxh_b = pp.tile([128, 512], BF16, tag="wxh_b")
            oh = pp.tile([128, (t_steps + UNROLL) * BS], BF16, tag="oh")
            ohwin = pp.tile([128, UNROLL * BS], BF16, tag="ohwin")
            fcwt_sb = pp.tile([128, 16 * V], BF16, tag="fcwt")
            fcb_row = pp.tile([1, V], BF16, tag="fcb")
            id_bf = pp.tile([128, 128], BF16, tag="ident")
            hta = pp.tile([128, 128], BF16, tag="hta")
            htb = pp.tile([128, 128], BF16, tag="htb")
            hga = pp.tile([128, 128], BF16, tag="hga")
            hgb = pp.tile([128, 128], BF16, tag="hgb")
            ones_bf = pp.tile([1, 128], BF16, tag="onesb")

            nc.gpsimd.memset(ones_bf[:], 1.0)
            nc.gpsimd.memset(hta[:], 0.0)  # h_{-1} = 0
            nc.gpsimd.memset(htb[:], 0.0)
            nc.gpsimd.memset(hga[:], 0.0)  # gap rows (32g+16..32g+32) stay 0
            nc.gpsimd.memset(hgb[:], 0.0)

            nc.gpsimd.dma_start(whh_a[:], whh_a_d[:])
            nc.gpsimd.dma_start(whh_b[:], whh_b_d[:])
            nc.gpsimd.dma_start(wxh_a[:], wxh_a_d[:])
            nc.gpsimd.dma_start(wxh_b[:], wxh_b_d[:])
            nc.gpsimd.dma_start(oh[:], oh_d[:])
            nc.gpsimd.dma_start(
                fcwt_sb.rearrange("p (c v) -> p c v", c=16)[:, :, :],
                fcwt_d.rearrange("(c p) v -> p c v", p=128)[:, :, :],
            )
            nc.gpsimd.dma_start(fcb_row[:], fcb_d[:])

            with tc.tile_pool(name="idtmp", bufs=1) as it_:
                id_f = it_.tile([128, 128], F32, tag="identf")
                make_identity(nc, id_f[:])
                nc.vector.tensor_copy(id_bf[:], id_f[:])

            hst_v = hst.rearrange("p (kb t) -> p kb t", t=t_steps)
            oh_w = oh.rearrange("p (t b) -> p t b", b=BS)
            ohwin_w = ohwin.rearrange("p (s b) -> p s b", b=BS)

            # ---- recurrence ----
            with (
                tc.tile_pool(name="psa", bufs=2, space="PSUM") as pap,
                tc.tile_pool(name="psb", bufs=2, space="PSUM") as pbp,
                tc.tile_pool(name="pst", bufs=2, space="PSUM") as ptp,
                tc.tile_pool(name="pdmy", bufs=1, space="PSUM") as pdmy_p,
            ):
                pdmy_m = pdmy_p.tile([1, 1], F32, tag="pdmym")
                pdmy_t = pdmy_p.tile([1, 1], BF16, tag="pdmyt")

                def pe_fence():
                    # dummy normal-mode matmul; absorbs PE mode-transition /
                    # structural self-waits so real matmuls keep a single
                    # sync wait (S3_LW codegen limit)
                    nc.tensor.matmul(
                        pdmy_m[:], id_bf[0:1, 0:1], id_bf[0:1, 0:1],
                        start=True, stop=True,
                    )

                def pe_fence_t():
                    nc.tensor.transpose(
                        pdmy_t[:], id_bf[0:1, 0:1], id_bf[0:1, 0:1]
                    )

                def step(t_expr, parity, s=None, iv=None):
                    ht_cur = hta if parity == 0 else htb
                    ht_new = htb if parity == 0 else hta
                    h_grp = hga if parity == 0 else hgb
                    if s is None:
                        # python-unrolled path: fixed window position
                        oh_t = ohwin_w[:, t_expr % UNROLL, :]
                    else:
                        oh_t = ohwin_w[:, s, :]  # fixed address (LDWEIGHTS)

                    pe_fence()
                    pa = pap.tile([128, 128], F32, tag="pa", name="pa")
                    pb = pbp.tile([128, 64], F32, tag="pb", name="pb")
                    # vocab first: no dependency on previous step's h.
                    # start=True clears the whole PSUM bank, so only the
                    # first matmul into pb carries it; later column slices
                    # land on has_written=0 cells (overwrite mode).
                    for jj in range(4):
                        nc.tensor.matmul(
                            pb[:, 16 * jj : 16 * jj + 16],
                            wxh_b[:, 128 * jj : 128 * jj + 128],
                            oh_t,
                            start=(jj == 0), stop=False,
                            skip_group_check=True,
                        )
                    for g in range(4):
                        nc.tensor.matmul(
                            pa[32 * g : 32 * g + 16, :],
                            oh_t,
                            wxh_a[:, 128 * g : 128 * g + 128],
                            start=True, stop=False,
                            tile_position=(0, 32 * g),
                        )
                    if s == UNROLL - 1 and iv is not None:
                        # prefetch next onehot window (single buffer; WAR on
                        # this window's vocab matmuls above keeps semantics)
                        nc.vector.tensor_copy(
                            ohwin_w[:, :, :],
                            oh_w[:, bass.DynSlice(iv + UNROLL, UNROLL), :],
                        )
                    # hidden: chunks 4-7 first (ready earliest: direct ACT of
                    # previous step), then 0-3 (via transpose + DVE copy)
                    for k in (4, 5, 6, 7, 0, 1, 2, 3):
                        last = k == 3
                        hk = ht_cur[:, 16 * k : 16 * k + 16]
                        for jj in range(4):
                            nc.tensor.matmul(
                                pb[:, 16 * jj : 16 * jj + 16],
                                whh_b[:, (jj * KCH + k) * 128 : (jj * KCH + k) * 128 + 128],
                                hk,
                                start=False, stop=last,
                            )
                        for g in range(4):
                            nc.tensor.matmul(
                                pa[32 * g : 32 * g + 16, :],
                                hk,
                                whh_a[:, k * 512 + 128 * g : k * 512 + 128 * g + 128],
                                start=False, stop=last,
                                tile_position=(0, 32 * g),
                            )
                    # tanh: part-b lands hidden-major directly in ht chunks 4-7
                    nc.scalar.activation(ht_new[:, 64:128], pb[:, :], AF.Tanh)
                    for g in range(4):
                        nc.scalar.activation(
                            h_grp[32 * g : 32 * g + 16, :],
                            pa[32 * g : 32 * g + 16, :],
                            AF.Tanh,
                        )
                    # part-a: one [128,128] transpose -> hidden-major chunks 0-3
                    pe_fence_t()
                    pt = ptp.tile([128, 128], BF16, tag="pt", name="pt")
                    nc.tensor.transpose(pt[:, :], h_grp[:, :], id_bf[:, :])
                    pt_v = pt.rearrange("p (g c) -> p g c", g=4)
                    htn_v = ht_new.rearrange("p (k b) -> p k b", k=KCH)
                    nc.vector.tensor_copy(htn_v[:, 0:4, :], pt_v[:, :, 0:BS])
                    # append h_t to HsT (off critical path)
                    nc.vector.tensor_copy(
                        hst_v[:, :, bass.ts(t_expr, 1)],
                        ht_new.rearrange("p (kb one) -> p kb one", one=1)[:, :, :],
                    )

                if t_steps <= 32:
                    for t in range(t_steps):
                        if t % UNROLL == 0:
                            nc.vector.tensor_copy(
                                ohwin_w[:, :, :],
                                oh_w[:, t : t + UNROLL, :],
                            )
                        step(t, t % 2)
                else:
                    assert t_steps % UNROLL == 0
                    nc.vector.tensor_copy(
                        ohwin_w[:, :, :], oh_w[:, 0:UNROLL, :]
                    )
                    with tc.For_i(
                        0, t_steps, UNROLL,
                        hint_engines=(mybir.EngineType.PE,),
                        staggered_reset=True,
                    ) as iv:
                        for s in range(UNROLL):
                            step(iv + s, s % 2, s=s, iv=iv)

            if hdbg_d is not None:
                nc.sync.dma_start(hdbg_d[:, :], hst[:, :])

            # ---- attention + fc, per batch ----
            with (
                tc.tile_pool(name="attn", bufs=1) as ap_,
                tc.tile_pool(name="attn2", bufs=2) as ap2,
                tc.tile_pool(name="psS", bufs=2, space="PSUM") as psS_p,
                tc.tile_pool(name="psG", bufs=2, space="PSUM") as psG_p,
                tc.tile_pool(name="ps1", bufs=2, space="PSUM") as ps1_p,
                tc.tile_pool(name="ps2", bufs=2, space="PSUM") as ps2_p,
            ):
                n_tc = t_steps // 128  # t-chunks of 128
                for b in range(BS):
                    def hs(k, sl):  # HsT tile for (k-chunk, slice of t)
                        return hst_v[:, k * BS + b, sl]

                    p_sb = ap_.tile([128, n_tc * t_steps], BF16, tag="p_sb")
                    rinv = ap_.tile([128, n_tc], F32, tag="rinv")
                    for c in range(n_tc):
                        psS = psS_p.tile([128, t_steps], F32, tag="psS")
                        for k in range(KCH):
                            nc.tensor.matmul(
                                psS[:],
                                hs(k, slice(128 * c, 128 * c + 128)),
                                hs(k, slice(0, t_steps)),
                                start=(k == 0),
                                stop=(k == KCH - 1),
                            )
                        rowsum = ap2.tile([128, 1], F32, tag="rowsum")
                        nc.scalar.activation(
                            p_sb[:, c * t_steps : (c + 1) * t_steps],
                            psS[:],
                            AF.Exp,
                            accum_out=rowsum[:],
                        )
                        nc.vector.reciprocal(rinv[:, c : c + 1], rowsum[:])
                    # G = Hs @ fc_w[:, H:].T  -> [t(=s) chunks, V]
                    g_sb = ap_.tile([128, n_tc * V], BF16, tag="g_sb")
                    for i in range(n_tc):
                        psG = psG_p.tile([128, V], F32, tag="psG")
                        for k in range(KCH):
                            nc.tensor.matmul(
                                psG[:],
                                hs(k, slice(128 * i, 128 * i + 128)),
                                fcwt_sb[:, (KCH + k) * V : (KCH + k + 1) * V],
                                start=(k == 0),
                                stop=(k == KCH - 1),
                            )
                        nc.vector.tensor_copy(g_sb[:, i * V : (i + 1) * V], psG[:])
                    # out[t-chunk c] = Hs@fc_wh.T + ones*fc_b + rinv*(P @ G)
                    for c in range(n_tc):
                        ps1 = ps1_p.tile([128, V], F32, tag="ps1")
                        for k in range(KCH):
                            nc.tensor.matmul(
                                ps1[:],
                                hs(k, slice(128 * c, 128 * c + 128)),
                                fcwt_sb[:, k * V : (k + 1) * V],
                                start=(k == 0),
                                stop=False,
                            )
                        nc.tensor.matmul(
                            ps1[:],
                            ones_bf[:],
                            fcb_row[:],
                            start=False,
                            stop=True,
                        )
                        ps2 = ps2_p.tile([128, V], F32, tag="ps2")
                        for i in range(n_tc):
                            # lhsT = P^T block (i,c) == P block, by symmetry
                            nc.tensor.matmul(
                                ps2[:],
                                p_sb[:, i * t_steps + 128 * c : i * t_steps + 128 * c + 128],
                                g_sb[:, i * V : (i + 1) * V],
                                start=(i == 0),
                                stop=(i == n_tc - 1),
                            )
                        o2 = ap2.tile([128, V], F32, tag="o2")
                        nc.vector.tensor_scalar_mul(o2[:], ps2[:], rinv[:, c : c + 1])
                        oo = ap2.tile([128, V], F32, tag="oo")
                        nc.vector.tensor_add(oo[:], ps1[:], o2[:])
                        nc.sync.dma_start(out_d[b, 128 * c : 128 * c + 128, :], oo[:])

    nc.compile()
    return nc


def _prep_core_inputs(inputs, core, t_steps=T):
    x = np.asarray(inputs["x"])[core * BS : (core + 1) * BS, :t_steps]
    wxhb = (
        np.asarray(inputs["Wxh"]).astype(np.float32)
        + np.asarray(inputs["bh"]).astype(np.float32)[None, :]
    )
    whh = np.asarray(inputs["Whh"]).astype(np.float32)
    w4 = whh.reshape(KCH, 128, KCH, 128)
    # whh_a[p, (k, g, j)] = Whh[128k+p, 128g+j], g < 4
    whh_a = np.ascontiguousarray(w4[:, :, :4, :].transpose(1, 0, 2, 3)).reshape(128, -1)
    # whh_b[p, (jj, k, j)] = Whh[128k+p, 512 + 128jj + j]
    whh_b = np.ascontiguousarray(w4[:, :, 4:, :].transpose(1, 2, 0, 3)).reshape(128, -1)
    # onehot: oh[v, 16t + b] = (x[b, t] == v); one zero window of padding
    oh = np.zeros((V, (t_steps + UNROLL) * BS), dtype=BFNP)
    oh[x.T.reshape(-1), np.arange(t_steps * BS)] = 1
    return {
        "whh_a": whh_a.astype(BFNP),
        "whh_b": whh_b.astype(BFNP),
        "wxh_a": np.ascontiguousarray(wxhb[:, :512]).astype(BFNP),
        "wxh_b": np.ascontiguousarray(wxhb[:, 512:]).astype(BFNP),
        "oh": oh,
        "fcwt": np.ascontiguousarray(
            np.asarray(inputs["fc_w"]).astype(np.float32).T
        ).astype(BFNP),
        "fcb": np.asarray(inputs["fc_b"]).astype(np.float32).reshape(1, V).astype(BFNP),
    }


def kernel(x, Wxh, Whh, bh, fc_w, fc_b, t_steps=T, trace=False):
    inputs = dict(x=x, Wxh=Wxh, Whh=Whh, bh=bh, fc_w=fc_w, fc_b=fc_b)
    nc = build_nc(t_steps)
    in_maps = [_prep_core_inputs(inputs, c, t_steps) for c in range(NCORES)]
    res = run_bass_kernel_spmd(nc, in_maps, core_ids=list(range(NCORES)), trace=trace)
    out = np.concatenate([r["out"] for r in res.results], axis=0)
    if trace:
        print(f"HW exec time: {res.exec_time_ns} ns", file=sys.stderr)
    return out
